# revision 1
# baseline (speedup 1.0000x reference)
"""GAT layer (gnn_message_passing) Trainium2 Bass kernel.

Reference computation (N=8192, F_IN=256, F_OUT=128):
    h   = x @ W
    e   = leakyrelu((h@a1)[:,None] + (h@a2)[None,:], 0.2)
    att = softmax(where(adj>0, e, -9e15), axis=1)
    out = elu(att @ h)

Distribution: 1D row-parallel over the node dim N across 8 cores
(rows i), each core holding the full column range j. All per-core
tensors live TRANSPOSED ([j, i] / [feat, i]) so the score matrix is
born in the layout the PE needs for att@h -- no on-device transposes
at all.

Per-core pipeline (i-block IB=1024, j-chunks of 128 partitions):
  stage A:       s128 = broadcast of s_i via (v1 (x) ones128) matmul;
                 t-row via v2 matmul + DRAM-bounce repartition to
                 tT[p, jc]; h_nat[j, feat] tiles = xT^T W  (all from
                 host-folded v1 = W@a1, v2 = W@a2)
  stage B per j-chunk (7 of 8 chunks, "route D"):
                 l3 = lrelu((mask + t_j) + s_i, 0.2) -- ONE fused
                 custom DVE op (GAT_SCORE_ANT, registered at import)
               every 8th chunk ("route A", relieves the DVE):
                 PSUM pre = Id8@mask + ones1@s_row  (PE)
                 l3 = Prelu(pre + t_j, alpha=0.2)   (ACT)
               then for all chunks:
                 p  = exp(l3 - 8)               (ACT, 4 chunks/op)
                 h'T += h_nat[jc]^T @ p         (PE, fp32 PSUM accum)
                 Zrep += ones128^T @ p          (PE, row-sum replicated
                                                 across partitions)
  stage C:       h' = h'T * recip(Z); elu(x) = max(x, exp(min(x,0))-1);
                 DMA out h'^T block (host re-transposes + concats)

Numerics: fp16 on the whole score path; the mask is fp8e4m3 in
{0, -240} applied as an *addition* before the leaky relu (commutes:
lrelu(e-240) ~ -48 -> exp(-48-8) == 0), so there is no separate mask
multiply pass and mask DMA traffic is 1 byte/element. exp is shifted
by a global constant 8 which cancels in the softmax ratio; h'/Z uses
fp32 PSUM accumulation. Validated absmax err ~3e-3 (0.13% of output
scale) vs the fp32 reference.
"""

import numpy as np

import concourse.bacc as bacc
import concourse.bass as bass
import concourse.mybir as mybir
import concourse.tile as tile
from concourse.alu_op_type import AluOpType
from concourse.bass_utils import run_bass_kernel_spmd

# problem constants (hardcoded per harness contract)
N = 8192
F_IN = 256
F_OUT = 128
N_CORES = 8
ALPHA = 0.2
NEG_MASK = -240.0    # fp8e4m3 max-negative finite; "minus infinity" for scores
EXP_SHIFT = -8.0     # global shift in exp(); cancels in softmax ratio

F16 = mybir.dt.float16
F32 = mybir.dt.float32
F8 = mybir.dt.float8e4

# ---- custom fused DVE op: l3 = lrelu((mask + t) + s, alpha) ----------------
# One 1x-mode DVE pass replaces two scalar_tensor_tensor passes (the
# dominant elementwise cost). Registered into concourse.dve_ops at import;
# the uop table is generated per-NEFF at compile time.
import concourse.dve_ops as _dve_ops
from concourse.dve_spec import Spec as _Spec, Src0 as _Src0, Src1 as _Src1, \
    C0 as _C0, C1 as _C1, maxx as _maxx, lower as _lower, _has_src1
from concourse.dve_uop import DveOpSpec as _DveOpSpec


def _register_gat_score_op():
    name = "GAT_SCORE_ANT"
    for op in _dve_ops.OPS:
        if op.name == name:
            return op
    y = (_Src0 + _C0) + _Src1
    spec = _Spec(
        body=_maxx(y, y * _C1),
        reference=lambda in0, in1, s0, s1, imm2: np.maximum(
            (in0 + s0) + in1, ((in0 + s0) + in1) * s1),
    )
    opcode = _dve_ops._CUSTOM_DVE_ROW_BASE + len(_dve_ops.OPS)
    assert opcode < 0x20
    shas = {}
    for ver in ("v3", "v4"):
        s = _DveOpSpec(name=name, opcode=opcode, uops=_lower(spec, ver=ver),
                       rd1_en=_has_src1(spec))
        shas[ver] = s.sha(ver)
    op = _dve_ops.DveOp(name, spec, subdim=False, uops_sha=shas)
    _dve_ops.OPS.append(op)
    _dve_ops._SUB_OPCODE_FOR_NAME[name] = opcode
    _dve_ops.CUSTOM_DVE_SPECS[name] = spec
    return op


GAT_SCORE = _register_gat_score_op()


def build_program(n=N, f_in=F_IN, f_out=F_OUT, ib=N // N_CORES, act_batch=4,
                  repeat=1, ablate=(), route_a_every=8, full_repeat=1):
    """Build the (SPMD, per-core identical) Bass program.

    n:    total node count (full j range handled by every core)
    ib:   i-block size owned by this core
    act_batch: j-chunks batched per ACT exp instruction
    repeat: run stage B this many times (benchmarking only; output
            unchanged since PSUM accumulation restarts each repeat)
    """
    assert n % 128 == 0 and f_in % 128 == 0 and f_out == 128
    njc = n // 128            # j-chunks
    nkc = f_in // 128         # f_in chunks
    nic = n // ib             # i-blocks across cores (for hT tiling)
    assert njc % act_batch == 0
    nq = njc // act_batch

    nc = bacc.Bacc("TRN2", target_bir_lowering=False, debug=False,
                   num_devices=N_CORES)

    mt = nc.dram_tensor("mt", [n, ib], F8, kind="ExternalInput").ap()
    xt = nc.dram_tensor("xt", [f_in, n], F16, kind="ExternalInput").ap()
    xtl = nc.dram_tensor("xtl", [f_in, ib], F16, kind="ExternalInput").ap()
    w = nc.dram_tensor("w", [f_in, f_out], F16, kind="ExternalInput").ap()
    a1r = nc.dram_tensor("a1r", [f_in, 128], F16, kind="ExternalInput").ap()
    a2c = nc.dram_tensor("a2c", [f_in, 1], F16, kind="ExternalInput").ap()
    id8 = nc.dram_tensor("id8", [128, 128], F8, kind="ExternalInput").ap()
    outT = nc.dram_tensor("outT", [f_out, ib], F32, kind="ExternalOutput").ap()

    with tile.TileContext(nc) as tc:
        for _fr in range(full_repeat):
            _kernel_body(tc, mt, xt, xtl, w, a1r, a2c, id8, outT,
                         n=n, f_in=f_in, f_out=f_out, ib=ib,
                         njc=njc, nkc=nkc, nic=nic, act_batch=act_batch,
                         nq=nq, repeat=repeat, ablate=frozenset(ablate),
                         route_a_every=route_a_every)
    nc.compile()
    return nc


def _kernel_body(tc, mt, xt, xtl, w, a1r, a2c, id8, outT, *,
                 n, f_in, f_out, ib, njc, nkc, nic, act_batch, nq, repeat=1,
                 ablate=frozenset(), route_a_every=8):
    nc = tc.nc
    AB = act_batch
    # hT column tiling: split the full j range into blocks of <=1024
    htb = min(1024, n)
    nhb = n // htb
    MMN = 512  # max matmul output free dim (one PSUM bank of fp32)

    def mm_split(out, lhsT, rhs, start, stop):
        """matmul with output free dim split into <=512 slices."""
        nfree = out.shape[-1]
        for o in range(0, nfree, MMN):
            sl = slice(o, min(o + MMN, nfree))
            nc.tensor.matmul(out[..., sl], lhsT=lhsT, rhs=rhs[..., sl],
                             start=start, stop=stop)

    from contextlib import ExitStack
    with ExitStack() as ctx:
        singles = ctx.enter_context(tc.tile_pool(name="singles", bufs=1))
        work = ctx.enter_context(tc.tile_pool(name="work", bufs=4))
        mtp = ctx.enter_context(tc.tile_pool(name="mtp", bufs=3))
        psA = ctx.enter_context(tc.tile_pool(name="psA", bufs=2, space="PSUM"))
        psM = ctx.enter_context(tc.tile_pool(name="psM", bufs=1, space="PSUM"))

        # ---- load constants (small tensors first so stage A starts early) --
        xtl_sb = singles.tile([128, nkc, ib], F16)
        nc.sync.dma_start(out=xtl_sb, in_=xtl.rearrange("(kc p) i -> p kc i", p=128))
        w_sb = singles.tile([128, nkc, f_out], F16)
        nc.sync.dma_start(out=w_sb, in_=w.rearrange("(kc p) m -> p kc m", p=128))
        v1r_sb = singles.tile([128, nkc, 128], F16)
        nc.sync.dma_start(out=v1r_sb, in_=a1r.rearrange("(kc p) m -> p kc m", p=128))
        v2c_sb = singles.tile([128, nkc, 1], F16)
        nc.sync.dma_start(out=v2c_sb, in_=a2c.rearrange("(kc p) m -> p kc m", p=128))
        xt_sb = singles.tile([128, nkc, n], F16)
        xt_r = xt.rearrange("(kc p) j -> p kc j", p=128)
        for hb in range(nhb):
            sl = slice(hb * htb, (hb + 1) * htb)
            nc.sync.dma_start(out=xt_sb[:, :, sl], in_=xt_r[:, :, sl])
        ones_sb = singles.tile([128, 128], F16)
        nc.vector.memset(ones_sb, 1.0)
        id8_sb = singles.tile([128, 128], F8)
        nc.sync.dma_start(out=id8_sb, in_=id8)
        shift_sb = singles.tile([128, 1], F32)
        nc.vector.memset(shift_sb, EXP_SHIFT)
        warm_sb = singles.tile([128, 1], F32)
        nc.scalar.activation(out=warm_sb, in_=shift_sb,
                             func=mybir.ActivationFunctionType.Exp,
                             bias=0.0, scale=1.0)

        # ---- stage A: s, t (via host-folded v1 = W@a1, v2 = W@a2) ----------
        # s128[p, i] = s_i = sum_f x^T[f,i] v1[f]  (v1 replicated as lhsT)
        s128_sb = singles.tile([128, ib], F16)
        ps = psA.tile([128, htb], F32, tag="pA")
        for kc in range(nkc):
            mm_split(ps[:, :ib], lhsT=v1r_sb[:, kc, :], rhs=xtl_sb[:, kc, :],
                     start=(kc == 0), stop=(kc == nkc - 1))
        nc.vector.tensor_copy(s128_sb, ps[:, :ib])

        # t-row[1, j] = sum_f v2[f] x^T[f, j]  (v2 as stationary, one LDW
        # per kc), then DMA-repartition [1, n] -> tT[p, jc] = t_{jc*128+p}
        tt_sb = singles.tile([128, njc], F32)
        tr_sb = singles.tile([1, n], F32)
        dram = ctx.enter_context(tc.tile_pool(name="dram", bufs=1, space="DRAM"))
        tr_dram = dram.tile([n], F32)
        for hb in range(nhb):
            ps_t = psA.tile([1, htb], F32, tag="pA")
            for kc in range(nkc):
                mm_split(ps_t, lhsT=v2c_sb[:, kc, :],
                         rhs=xt_sb[:, kc, hb * htb:(hb + 1) * htb],
                         start=(kc == 0), stop=(kc == nkc - 1))
            sl = slice(hb * htb, (hb + 1) * htb)
            nc.scalar.copy(tr_sb[:, sl], ps_t)
            nc.scalar.dma_start(out=tr_dram[sl], in_=tr_sb[:, sl])
            njc_hb = htb // 128
            dst = tt_sb[:, hb * njc_hb:(hb + 1) * njc_hb]
            src_ap = tr_dram[sl].rearrange("(jc p) -> p jc", p=128)
            nc.scalar.dma_start(out=dst, in_=src_ap)

        # h_nat[j, feat] tiles (lhsT for att matmul): xT chunk as lhsT, W as rhs
        hn_sb = singles.tile([128, njc, f_out], F16)
        hn_group = min(1024 // f_out, njc)  # j-chunks per psum tile
        for jg in range(njc // hn_group):
            ps = psA.tile([128, hn_group, f_out], F32, tag="pA")
            for g in range(hn_group):
                jc = jg * hn_group + g
                for kc in range(nkc):
                    nc.tensor.matmul(ps[:, g, :],
                                     lhsT=xt_sb[:, kc, jc * 128:(jc + 1) * 128],
                                     rhs=w_sb[:, kc, :],
                                     start=(kc == 0), stop=(kc == nkc - 1))
            dst = hn_sb[:, jg * hn_group:(jg + 1) * hn_group, :]
            if jg % 2 == 0:
                nc.scalar.copy(dst, ps)
            else:
                nc.vector.tensor_copy(dst, ps)

        # ---- stage B: masked softmax numerator + att matmul ----------------
        ps_hpT = psM.tile([128, ib], F32, tag="hpT")
        ps_z = psM.tile([128, ib], F32, tag="z")

        for rep in range(repeat):
          for jq in range(nq):
            mt_q = mtp.tile([128, AB, ib], F8, tag="mt")
            if "dma" not in ablate:
                nc.gpsimd.dma_start(
                    out=mt_q,
                    in_=mt.rearrange("(jq q p) i -> jq p q i", q=AB, p=128)[jq])
            else:
                nc.vector.memset(mt_q[:, 0, 0:1], 0.0)
            l3_q = work.tile([128, AB, ib], F16, tag="l3")
            if "dve" not in ablate:
                for q in range(AB):
                    jc = jq * AB + q
                    if route_a_every and jc % route_a_every == (
                            route_a_every - 1):
                        # route A: PE builds mask+s in PSUM, ACT does
                        # lrelu(.+t) -- relieves the DVE bottleneck
                        pre3 = psA.tile([128, ib], F32, tag="pA")
                        mm_split(pre3, lhsT=id8_sb, rhs=mt_q[:, q, :],
                                 start=True, stop=False)
                        mm_split(pre3, lhsT=ones_sb[0:1, :],
                                 rhs=s128_sb[0:1, :],
                                 start=False, stop=True)
                        nc.scalar.activation(
                            out=l3_q[:, q, :], in_=pre3,
                            func=mybir.ActivationFunctionType.Prelu,
                            bias=tt_sb[:, jc:jc + 1], scale=1.0, alpha=ALPHA)
                    else:
                        # route D: one fused DVE op
                        # l3 = lrelu((mask + t_j) + s_i, 0.2)
                        nc.vector._custom_dve(
                            GAT_SCORE, out=l3_q[:, q, :], in0=mt_q[:, q, :],
                            in1=s128_sb, s0=tt_sb[:, jc:jc + 1], s1=ALPHA)
            else:
                nc.vector.memset(l3_q[:, 0, 0:1], 0.0)
            if "act" not in ablate:
                p_q = work.tile([128, AB, ib], F16, tag="p")
                if jq == 0 or jq == nq - 1:
                    # first/last quad: per-chunk exp so downstream matmuls
                    # start (head) / drain (tail) one chunk at a time
                    for q in range(AB):
                        nc.scalar.activation(
                            out=p_q[:, q, :], in_=l3_q[:, q, :],
                            func=mybir.ActivationFunctionType.Exp,
                            bias=shift_sb, scale=1.0)
                else:
                    nc.scalar.activation(out=p_q, in_=l3_q,
                                         func=mybir.ActivationFunctionType.Exp,
                                         bias=shift_sb, scale=1.0)
            else:
                p_q = l3_q
            if "pe" not in ablate:
                for q in range(AB):
                    jc = jq * AB + q
                    # tensor-major: one h_nat and one ones LDWEIGHTS per
                    # chunk (slice-major doubles PE weight reloads)
                    mm_split(ps_hpT, lhsT=hn_sb[:, jc, :], rhs=p_q[:, q, :],
                             start=(jc == 0), stop=(jc == njc - 1))
                    mm_split(ps_z, lhsT=ones_sb, rhs=p_q[:, q, :],
                             start=(jc == 0), stop=(jc == njc - 1))
            elif jq == nq - 1 and rep == repeat - 1:
                nc.tensor.matmul(ps_hpT[:, 0:1], lhsT=hn_sb[:, 0, :],
                                 rhs=p_q[:, 0, 0:1], start=True, stop=True)
                nc.tensor.matmul(ps_z[:, 0:1], lhsT=ones_sb,
                                 rhs=p_q[:, 0, 0:1], start=True, stop=True)

        # ---- stage C: normalize + ELU + store (two i-halves, overlapped) ---
        z_sb = singles.tile([128, ib], F32)
        zr_sb = singles.tile([128, ib], F32)
        scratch = singles.tile([128, ib], F32)
        hn_f = singles.tile([128, ib], F32)
        m0 = singles.tile([128, ib], F32)
        expm = singles.tile([128, ib], F32)
        elu_sb = singles.tile([128, ib], F32)
        hb2 = ib // 2
        for h in range(2):
            sl = slice(h * hb2, (h + 1) * hb2)
            nc.vector.reciprocal_approx_accurate(zr_sb[:, sl], ps_z[:, sl],
                                                 scratch[:, sl])
            nc.vector.tensor_tensor(out=hn_f[:, sl], in0=ps_hpT[:, sl],
                                    in1=zr_sb[:, sl], op=AluOpType.mult)
            nc.vector.tensor_scalar_min(m0[:, sl], hn_f[:, sl], 0.0)
            nc.scalar.activation(out=expm[:, sl], in_=m0[:, sl],
                                 func=mybir.ActivationFunctionType.Exp,
                                 bias=0.0, scale=1.0)
            # elu = max(hn, exp(min(hn,0)) - 1)
            nc.vector.scalar_tensor_tensor(
                out=elu_sb[:, sl], in0=expm[:, sl], scalar=-1.0,
                in1=hn_f[:, sl], op0=AluOpType.add, op1=AluOpType.max)
            nc.sync.dma_start(out=outT[:, sl], in_=elu_sb[:, sl])


def prep_inputs(x, adj, W, a, n=N, ib=N // N_CORES):
    """Host-side sharding/layout prep. Returns list of per-core in_maps."""
    f16 = np.float16
    f8 = mybir.dt.np(F8)
    n_cores = n // ib
    xT = np.ascontiguousarray(x.T.astype(f16))
    Wh = np.ascontiguousarray(W.astype(f16))
    # fold the attention vectors through W on the host (weight prep):
    # s = h@a1 = x@(W@a1), t = h@a2 = x@(W@a2)
    v1 = (W.astype(np.float64) @ a[:F_OUT, 0].astype(np.float64)).astype(f16)
    v2 = (W.astype(np.float64) @ a[F_OUT:, 0].astype(np.float64)).astype(f16)
    A1rep = np.ascontiguousarray(np.tile(v1[:, None], (1, 128)))
    a2col = np.ascontiguousarray(v2[:, None])
    id8_np = np.eye(128, dtype=f8)
    in_maps = []
    for c in range(n_cores):
        i0 = c * ib
        blk = adj[i0:i0 + ib, :]  # [ib, n]
        mt_c = np.where(blk.T > 0, np.float32(0),
                        np.float32(NEG_MASK)).astype(f8)  # [n, ib]
        in_maps.append({
            "id8": id8_np,
            "mt": np.ascontiguousarray(mt_c),
            "xt": xT,
            "xtl": np.ascontiguousarray(xT[:, i0:i0 + ib]),
            "w": Wh,
            "a1r": A1rep,
            "a2c": a2col,
        })
    return in_maps


_CACHED_NC = None


def kernel(x, adj, W, a):
    global _CACHED_NC
    if _CACHED_NC is None:
        _CACHED_NC = build_program()
    nc = _CACHED_NC
    in_maps = prep_inputs(np.asarray(x), np.asarray(adj),
                          np.asarray(W), np.asarray(a))
    res = run_bass_kernel_spmd(nc, in_maps, core_ids=list(range(N_CORES)))
    blocks = [np.ascontiguousarray(res.results[c]["outT"].T)
              for c in range(N_CORES)]
    return np.concatenate(blocks, axis=0).astype(np.float32)

